# revision 14
# baseline (speedup 1.0000x reference)
"""Trainium2 Bass kernel for nn_Decoder: 2-layer GRU decoder, batch-parallel over 8 cores.

v3 design:
  - Shard batch 128 -> 16 rows/core, replicate weights (SBUF-resident).
  - Weight matmuls batch-major (weights are the moving operand, N=512) in
    fp8e4 DoubleRow mode (2 k-tiles per matmul, 0.5 cyc/row) with weights
    pre-scaled by SCALE=64; biases and the step-invariant glob@Wg term are
    accumulated into PSUM by id16 / ones-row matmuls, so the only
    activation-scale fixup is scale=1/SCALE inside sigmoid/tanh.
  - Gate math runs in TRANSPOSED space (gates on 128 partitions): gate psums
    [16,512] are copied to SBUF fp16 (spread over ACT/DVE/Pool), transposed
    on the PE (is_transpose), and the elementwise GRU update runs on
    [128, 64] tiles. The updated hidden state is produced directly in the
    transposed layout needed as the next matmul's stationary operand - the
    recurrence loop contains no DMA except the per-step output row.
  - fp32 transposed master states; fp8 shadows for the gate matmuls; fp16
    shadow of h1 for the (unscaled) fc head.
  - Software-pipelined emission per step so the in-order PE queue never
    waits long on the vector engines.
"""
import sys

sys.path.insert(0, "/opt/trn_rl_repo")
import numpy as np

import concourse.bass as bass
import concourse.mybir as mybir
import concourse.tile as tile
from concourse import bacc
from concourse.bass import ds, ts
from concourse.bass_utils import run_bass_kernel_spmd

F8 = mybir.dt.float8e4
F16 = mybir.dt.float16
F32 = mybir.dt.float32
AF = mybir.ActivationFunctionType
DR = mybir.MatmulPerfMode.DoubleRow

BS, H, D, SEQ = 128, 1024, 128, 256
NCORES = 8
B = BS // NCORES          # 16 rows per core
KH = H // 128             # 8 k-tiles over hidden dim
KP = KH // 2              # 4 k-PAIRS for fp8 DoubleRow
G3 = 3 * H                # 3072 gate cols
EMBED_DIM = 265216
TS_OFF = 3 * H
TS_LEN = SEQ * (H // 2)   # 131072
SCALE = 64.0              # fp8 weight pre-scale
INV = 1.0 / SCALE


def build_nc(n_steps=SEQ, unroll=15, static=False):
    nc = bacc.Bacc()

    d_embed = nc.declare_dram_parameter("embed", [B, EMBED_DIM], F32, isOutput=False)
    d_x0 = nc.declare_dram_parameter("x0", [B, D], F32, isOutput=False)
    d_wx0 = nc.declare_dram_parameter("wx0", [128, G3], F16, isOutput=False)
    d_wg0 = nc.declare_dram_parameter("wg0", [128, KH, G3], F16, isOutput=False)
    d_whh0 = nc.declare_dram_parameter("whh0", [128, KH, G3], F8, isOutput=False)
    d_wih1 = nc.declare_dram_parameter("wih1", [128, KH, G3], F8, isOutput=False)
    d_whh1 = nc.declare_dram_parameter("whh1", [128, KH, G3], F8, isOutput=False)
    d_fct = nc.declare_dram_parameter("fct", [128, KH, D], F16, isOutput=False)
    d_pred = nc.declare_dram_parameter("predt", [128, 4, D], F16, isOutput=False)
    d_s0b = nc.declare_dram_parameter("s0b", [G3], F16, isOutput=False)
    d_s1tT = nc.declare_dram_parameter("s1tT", [128, 3, KH, B], F16, isOutput=False)
    d_bh0nT = nc.declare_dram_parameter("bh0nT", [128, KH, B], F16, isOutput=False)
    d_bh1nT = nc.declare_dram_parameter("bh1nT", [128, KH, B], F16, isOutput=False)
    d_fcb = nc.declare_dram_parameter("fcb", [D, 1], F32, isOutput=False)
    d_pb2 = nc.declare_dram_parameter("pb2", [D], F32, isOutput=False)
    d_id16 = nc.declare_dram_parameter("id16", [B, B], F16, isOutput=False)
    d_id128 = nc.declare_dram_parameter("id128", [128, 128], F16, isOutput=False)
    d_ones = nc.declare_dram_parameter("ones16", [1, B], F16, isOutput=False)
    d_out = nc.declare_dram_parameter("out", [B, SEQ, D], F32, isOutput=True)

    # DRAM scratch for init-time transposes and the tail
    d_bh0 = nc.dram_tensor("bh0", [B, H], F16)
    d_bh1 = nc.dram_tensor("bh1", [B, H], F16)
    d_bg = nc.dram_tensor("bg", [B, H], F16)
    d_bx = nc.dram_tensor("bx", [B, D], F16)
    d_bts = nc.dram_tensor("bts", [128, 512], F16)

    def bcast(ap_1d, parts, n):
        return bass.AP(tensor=ap_1d.tensor, offset=ap_1d.offset,
                       ap=[[0, parts]] + list(ap_1d.ap))

    with tile.TileContext(nc) as tc:
        with (
            tc.tile_pool(name="persist", bufs=1) as pp,
            tc.tile_pool(name="tmp", bufs=2) as tp,
            tc.tile_pool(name="psum", bufs=8, space="PSUM") as qq,
        ):
            # ---------------- resident tiles ----------------
            s_wx0 = pp.tile([128, G3], F16)
            s_whh0 = pp.tile([128, KH, G3], F8)
            s_wih1 = pp.tile([128, KH, G3], F8)
            s_whh1 = pp.tile([128, KH, G3], F8)
            s_fct = pp.tile([128, KH, D], F16)
            s_pred = pp.tile([128, 4, D], F16)
            s_s0 = pp.tile([B, G3], F16)      # (glob@Wg + biases) * SCALE
            s_s0T = pp.tile([128, 3, KH, B], F16)  # transposed S0 (r,z,n)
            s_s1T = pp.tile([128, 3, KH, B], F16)  # transposed L1 biases
            s_bh0nT = pp.tile([128, KH, B], F16)   # b_hh0 n-part, transposed
            s_bh1nT = pp.tile([128, KH, B], F16)
            s_fcb = pp.tile([D, 1], F32)
            s_pb2 = pp.tile([128, D], F32)
            s_id16 = pp.tile([B, B], F16)
            s_id128 = pp.tile([128, 128], F16)
            # transposed states
            s_h0tm = pp.tile([128, KH, B], F32)   # master
            s_h1tm = pp.tile([128, KH, B], F32)
            s_h0t8 = pp.tile([128, KH, B], F8)    # matmul operands
            s_h1t8 = pp.tile([128, KH, B], F8)
            s_h1t16 = pp.tile([128, KH, B], F16)  # fc operand (unscaled fp16)
            s_xt = pp.tile([128, B], F16)
            s_xo = pp.tile([B, D], F32)

            nc.sync.dma_start(out=s_wx0, in_=d_wx0[:, :])
            nc.sync.dma_start(out=s_whh0, in_=d_whh0[:, :, :])
            nc.sync.dma_start(out=s_wih1, in_=d_wih1[:, :, :])
            nc.sync.dma_start(out=s_whh1, in_=d_whh1[:, :, :])
            nc.sync.dma_start(out=s_fct, in_=d_fct[:, :, :])
            nc.sync.dma_start(out=s_pred, in_=d_pred[:, :, :])
            nc.gpsimd.dma_start(out=s_s0, in_=bcast(d_s0b[:], B, G3))
            nc.sync.dma_start(out=s_s1T, in_=d_s1tT[:, :, :, :])
            nc.sync.dma_start(out=s_bh0nT, in_=d_bh0nT[:, :, :])
            nc.sync.dma_start(out=s_bh1nT, in_=d_bh1nT[:, :, :])
            nc.sync.dma_start(out=s_fcb, in_=d_fcb[:, :])
            nc.gpsimd.dma_start(out=s_pb2, in_=bcast(d_pb2[:], 128, D))
            nc.sync.dma_start(out=s_id16, in_=d_id16[:, :])
            nc.sync.dma_start(out=s_id128, in_=d_id128[:, :])

            # ---- initial transposed states (fp16 via DRAM bounce) ----
            s_hi0 = tp.tile([B, H], F32, tag="bigtmp")
            nc.sync.dma_start(out=s_hi0, in_=d_embed[:, H:2 * H])
            s_hi0h = tp.tile([B, H], F16, tag="bigtmp")
            nc.scalar.activation(s_hi0h, s_hi0, AF.Copy)
            nc.sync.dma_start(out=d_bh0[:, :], in_=s_hi0h)
            s_hi1 = tp.tile([B, H], F32, tag="bigtmp")
            nc.sync.dma_start(out=s_hi1, in_=d_embed[:, 2 * H:3 * H])
            s_hi1h = tp.tile([B, H], F16, tag="bigtmp")
            nc.scalar.activation(s_hi1h, s_hi1, AF.Copy)
            nc.sync.dma_start(out=d_bh1[:, :], in_=s_hi1h)
            s_h0ti = tp.tile([128, KH, B], F16, tag="bigtmp")
            nc.sync.dma_start_transpose(s_h0ti[:], d_bh0[:, :])
            s_h1ti = tp.tile([128, KH, B], F16, tag="bigtmp")
            nc.sync.dma_start_transpose(s_h1ti[:], d_bh1[:, :])
            nc.vector.tensor_copy(s_h0tm, s_h0ti)
            nc.vector.tensor_copy(s_h1tm, s_h1ti)
            nc.scalar.activation(s_h0t8, s_h0ti, AF.Copy)
            nc.scalar.activation(s_h1t8, s_h1ti, AF.Copy)
            nc.vector.tensor_copy(s_h1t16, s_h1ti)

            # x0 -> xT (fp16)
            s_x0f = tp.tile([B, D], F32, tag="bigtmp")
            nc.sync.dma_start(out=s_x0f, in_=d_x0[:, :])
            s_x0h = tp.tile([B, D], F16, tag="bigtmp")
            nc.scalar.activation(s_x0h, s_x0f, AF.Copy)
            nc.sync.dma_start(out=d_bx[:, :], in_=s_x0h)
            s_xtT = tp.tile([128, 1, B], F16, tag="bigtmp")
            nc.sync.dma_start_transpose(s_xtT[:], d_bx[:, :])
            nc.vector.tensor_copy(s_xt, s_xtT.rearrange("p o b -> p (o b)"))

            # glob -> globT; s_s0 += SCALE * glob @ Wg0 (wg0 pre-scaled)
            s_gf = tp.tile([B, H], F32, tag="wg", bufs=1)
            nc.sync.dma_start(out=s_gf, in_=d_embed[:, 0:H])
            s_gh = tp.tile([B, H], F16, tag="bigtmp")
            nc.scalar.activation(s_gh, s_gf, AF.Copy)
            nc.sync.dma_start(out=d_bg[:, :], in_=s_gh)
            s_gT = tp.tile([128, KH, B], F16, tag="bigtmp")
            nc.sync.dma_start_transpose(s_gT[:], d_bg[:, :])
            NCH = G3 // 512
            pg = [qq.tile([B, 512], F32, tag="ps", name=f"pg{c}") for c in range(NCH)]
            for k in range(KH):
                wbuf = tp.tile([128, G3], F16, tag="wg", bufs=1)
                nc.sync.dma_start(out=wbuf, in_=d_wg0[:, k, :])
                for c in range(NCH):
                    nc.tensor.matmul(pg[c], s_gT[:, k, :], wbuf[:, ts(c, 512)],
                                     start=(k == 0), stop=(k == KH - 1))
            for c in range(NCH):
                nc.vector.tensor_add(s_s0[:, ts(c, 512)], pg[c], s_s0[:, ts(c, 512)])

            # ---------------- one recurrence step ----------------
            # column slices for half h (h in 0,1)
            def slr(h):
                return ts(h, 512)

            def slz(h):
                return slice(H + h * 512, H + (h + 1) * 512)

            def sln(h):
                return slice(2 * H + h * 512, 2 * H + (h + 1) * 512)

            def dr_chain(p, sht8, w, colsl, start, stop):
                # fp8 DoubleRow accumulation of w.T @ h over 4 k-pairs
                for kp in range(KP):
                    nc.tensor.matmul(p, sht8[:, 2 * kp:2 * kp + 2, :],
                                     w[:, 2 * kp:2 * kp + 2, colsl],
                                     start=(start and kp == 0),
                                     stop=(stop and kp == KP - 1), perf_mode=DR)

            def l0_gh(h, tag):
                """gh0 for r,z,ghn of half h (only needs old h0t8)."""
                p_r = qq.tile([B, 512], F32, tag="ps", name=f"{tag}r")
                dr_chain(p_r, s_h0t8, s_whh0, slr(h), True, False)
                p_z = qq.tile([B, 512], F32, tag="ps", name=f"{tag}z")
                dr_chain(p_z, s_h0t8, s_whh0, slz(h), True, False)
                p_gh = qq.tile([B, 512], F32, tag="ps", name=f"{tag}gh")
                dr_chain(p_gh, s_h0t8, s_whh0, sln(h), True, False)
                nc.tensor.matmul(p_gh, s_ones, s_bh0r[:, ts(h, 512)],
                                 start=False, stop=True)
                return p_r, p_z, p_gh

            def l0_gix(h, p_r, p_z, tag):
                """x@Wx + S0 contributions of half h (needs s_xt)."""
                nc.tensor.matmul(p_r, s_xt, s_wx0[:, slr(h)], start=False, stop=False)
                nc.tensor.matmul(p_r, s_id16, s_s0[:, slr(h)], start=False, stop=True)
                nc.tensor.matmul(p_z, s_xt, s_wx0[:, slz(h)], start=False, stop=False)
                nc.tensor.matmul(p_z, s_id16, s_s0[:, slz(h)], start=False, stop=True)
                p_gi = qq.tile([B, 512], F32, tag="ps", name=f"{tag}gi")
                nc.tensor.matmul(p_gi, s_xt, s_wx0[:, sln(h)], start=True, stop=False)
                nc.tensor.matmul(p_gi, s_id16, s_s0[:, sln(h)], start=False, stop=True)
                return p_gi

            def l1_gh(h, tag):
                """gh1 for r,z,ghn of half h (only needs old h1t8)."""
                p_r = qq.tile([B, 512], F32, tag="ps", name=f"{tag}r")
                dr_chain(p_r, s_h1t8, s_whh1, slr(h), True, False)
                p_z = qq.tile([B, 512], F32, tag="ps", name=f"{tag}z")
                dr_chain(p_z, s_h1t8, s_whh1, slz(h), True, False)
                p_gh = qq.tile([B, 512], F32, tag="ps", name=f"{tag}gh")
                dr_chain(p_gh, s_h1t8, s_whh1, sln(h), True, False)
                nc.tensor.matmul(p_gh, s_ones, s_bh1r[:, ts(h, 512)],
                                 start=False, stop=True)
                return p_r, p_z, p_gh

            def l1_gi(h, p_r, p_z, tag):
                """c0@Wih1 + S1 contributions of half h (needs NEW h0t8)."""
                dr_chain(p_r, s_h0t8, s_wih1, slr(h), False, False)
                nc.tensor.matmul(p_r, s_id16, s_s1[:, slr(h)], start=False, stop=True)
                dr_chain(p_z, s_h0t8, s_wih1, slz(h), False, False)
                nc.tensor.matmul(p_z, s_id16, s_s1[:, slz(h)], start=False, stop=True)
                p_gi = qq.tile([B, 512], F32, tag="ps", name=f"{tag}gi")
                dr_chain(p_gi, s_h0t8, s_wih1, sln(h), True, False)
                nc.tensor.matmul(p_gi, s_id16, s_s1[:, sln(h)], start=False, stop=True)
                return p_gi

            def post(h, p_r, p_z, p_gi, p_gh, s_htm, s_ht8, extra16):
                """psum->sbuf copies, PE transposes, transposed gate math for
                half h of one layer. Updates master + fp8 (+fp16) states."""
                # psum -> sbuf fp16 copies, spread over ACT/DVE/Pool
                c_r = tp.tile([B, 512], F16, tag="cp", bufs=8, name="c_r")
                c_z = tp.tile([B, 512], F16, tag="cp", bufs=8, name="c_z")
                c_gi = tp.tile([B, 512], F16, tag="cp", bufs=8, name="c_gi")
                c_gh = tp.tile([B, 512], F16, tag="cp", bufs=8, name="c_gh")
                nc.vector.tensor_copy(c_r, p_r)
                nc.scalar.activation(c_z, p_z, AF.Copy)
                nc.vector.tensor_copy(c_gi, p_gi)
                nc.scalar.activation(c_gh, p_gh, AF.Copy)
                # PE transposes: [16,128] chunks -> [128,16]; kinds r,z,gi,gh
                pT = qq.tile([128, 4, 4, B], F16, tag="ps", name="pT")
                for ki, src in enumerate((c_r, c_z, c_gi, c_gh)):
                    for q in range(4):
                        nc.tensor.transpose(pT[:, ki, q, :],
                                            src[:, q * 128:(q + 1) * 128], s_id16)
                # transposed gate math on [128, 4*B] tiles
                rz = tp.tile([128, 2, 4, B], F16, tag="rz", bufs=4)
                nc.scalar.activation(rz, pT[:, 0:2], AF.Sigmoid, scale=INV)
                tn = tp.tile([128, 4, B], F32, tag="tn", bufs=4)
                nc.vector.tensor_mul(tn, rz[:, 0], pT[:, 3])
                nc.vector.tensor_add(tn, tn, pT[:, 2])
                nc.scalar.activation(tn, tn, AF.Tanh, scale=INV)
                m = s_htm[:, 4 * h:4 * h + 4, :]
                td = tp.tile([128, 4, B], F32, tag="td", bufs=4)
                nc.vector.tensor_sub(td, m, tn)
                nc.vector.tensor_mul(td, rz[:, 1], td)
                nc.vector.tensor_add(m, tn, td)
                nc.scalar.activation(s_ht8[:, 4 * h:4 * h + 4, :], m, AF.Copy)
                if extra16 is not None:
                    nc.vector.tensor_copy(extra16[:, 4 * h:4 * h + 4, :], m)

            def fc_block(t_expr):
                pfcT = qq.tile([D, B], F32, tag="ps", name="pfcT")
                for k in range(KH):
                    nc.tensor.matmul(pfcT, s_fct[:, k, :], s_h1t16[:, k, :],
                                     start=(k == 0), stop=(k == KH - 1))
                nc.scalar.activation(s_xt, pfcT, AF.Sigmoid, bias=s_fcb[:, :])
                pfcA = qq.tile([B, D], F32, tag="ps", name="pfcA")
                for k in range(KH):
                    nc.tensor.matmul(pfcA, s_h1t16[:, k, :], s_fct[:, k, :],
                                     start=(k == 0), stop=(k == KH - 1))
                nc.vector.tensor_add(s_xo, pfcA, s_fcbrow)
                nc.scalar.activation(s_xo, s_xo, AF.Sigmoid)
                nc.sync.dma_start(out=d_out[:, ds(t_expr, 1), :],
                                  in_=s_xo.rearrange("b d -> b () d"))

            def step(t_expr, first):
                # L0 gh matmuls (old h0) keep the PE busy while the previous
                # step's L1 gate math finishes on the vector engines.
                r0a, z0a, gh0a = l0_gh(0, "a")
                r0b, z0b, gh0b = l0_gh(1, "b")
                # previous step's fc -> s_xt for this step + output row
                if not first:
                    fc_block(t_expr - 1)
                gi0a = l0_gix(0, r0a, z0a, "a")
                gi0b = l0_gix(1, r0b, z0b, "b")
                # L1 gh matmuls (old h1) cover the L0 copies/gate math
                r1a, z1a, gh1a = l1_gh(0, "c")
                r1b, z1b, gh1b = l1_gh(1, "d")
                post(0, r0a, z0a, gi0a, gh0a, s_h0tm, s_h0t8, None)
                post(1, r0b, z0b, gi0b, gh0b, s_h0tm, s_h0t8, None)
                # L1 gi matmuls contract the NEW h0t8
                gi1a = l1_gi(0, r1a, z1a, "c")
                gi1b = l1_gi(1, r1b, z1b, "d")
                post(0, r1a, z1a, gi1a, gh1a, s_h1tm, s_h1t8, s_h1t16)
                post(1, r1b, z1b, gi1b, gh1b, s_h1tm, s_h1t8, s_h1t16)

            if static:
                for t in range(n_steps):
                    step(t, t == 0)
            else:
                step(0, True)
                while (n_steps - 1) % unroll != 0:
                    unroll -= 1
                with tc.For_i(1, n_steps, unroll,
                              hint_engines=(mybir.EngineType.PE,)) as iv:
                    for j in range(unroll):
                        step(iv + j, False)
            fc_block(n_steps - 1)

            # ---------------- tail: trend/season + residual ----------------
            for b in range(B):
                for si in range(2):
                    base = TS_OFF + si * 128 * 512
                    ps_o = qq.tile([128, D], F32, tag="ps")
                    for which in range(2):  # 0=trend 1=season
                        off = base + which * TS_LEN
                        src = d_embed[b:b + 1, off:off + 65536].rearrange(
                            "o (s f) -> (o s) f", f=512)
                        t_f = tp.tile([128, 512], F32, tag="tsf")
                        nc.sync.dma_start(out=t_f, in_=src)
                        t_h = tp.tile([128, 512], F16, tag="bigtmp")
                        nc.scalar.activation(t_h, t_f, AF.Copy)
                        # PE transpose [128,128] chunks (f on partitions)
                        p_tT = qq.tile([128, 4, 128], F16, tag="ps", name="p_tT")
                        for jj in range(4):
                            nc.tensor.transpose(p_tT[:, jj, :],
                                                t_h[:, jj * 128:(jj + 1) * 128],
                                                s_id128)
                        t_T = tp.tile([128, 4, 128], F16, tag="bigtmp")
                        nc.vector.tensor_copy(t_T, p_tT)
                        for jj in range(4):
                            nc.tensor.matmul(ps_o, t_T[:, jj, :], s_pred[:, jj, :],
                                             start=(which == 0 and jj == 0),
                                             stop=(which == 1 and jj == 3))
                    r_c = tp.tile([128, D], F32, tag="bigtmp")
                    nc.sync.dma_start(out=r_c, in_=d_out[b, si * 128:(si + 1) * 128, :])
                    nc.vector.tensor_add(r_c, ps_o, r_c)
                    nc.vector.tensor_add(r_c, r_c, s_pb2)
                    nc.sync.dma_start(out=d_out[b, si * 128:(si + 1) * 128, :], in_=r_c)

    nc.compile()
    return nc


def _prep_weights(W_ih0, W_hh0, b_ih0, b_hh0, W_ih1, W_hh1, b_ih1, b_hh1,
                  fc_W, fc_b, pred_W, pred_b):
    f16 = np.float16
    f8 = mybir.dt.np(F8)

    def karr(WT, dt, scale=1.0):  # [K, N] -> [128, K/128, N]
        K, N = WT.shape
        return np.ascontiguousarray(
            (WT * scale).reshape(K // 128, 128, N).transpose(1, 0, 2)).astype(dt)

    return dict(
        wx0=(np.ascontiguousarray(W_ih0[:, H:H + D].T) * SCALE).astype(f16),
        wg0=karr(W_ih0[:, :H].T, f16, SCALE),
        whh0=karr(W_hh0.T, f8, SCALE),
        wih1=karr(W_ih1.T, f8, SCALE),
        whh1=karr(W_hh1.T, f8, SCALE),
        fct=karr(fc_W.T, f16),
        predt=np.ascontiguousarray(
            pred_W.T.reshape(4, 128, D).transpose(1, 0, 2)).astype(f16),
        s0b=(np.concatenate([(b_ih0 + b_hh0)[:2 * H], b_ih0[2 * H:]]) * SCALE).astype(f16),
        s1b=(np.concatenate([(b_ih1 + b_hh1)[:2 * H], b_ih1[2 * H:]]) * SCALE).astype(f16),
        bhh0n=(b_hh0[2 * H:] * SCALE).astype(f16),
        bhh1n=(b_hh1[2 * H:] * SCALE).astype(f16),
        fcb=np.ascontiguousarray(fc_b.reshape(D, 1)).astype(np.float32),
        id16=np.eye(B, dtype=np.float16),
        id128=np.eye(128, dtype=np.float16),
        ones16=np.ones((1, B), dtype=np.float16),
        pb2=(2.0 * pred_b).astype(np.float32),
    )


_NC_CACHE = {}


def kernel(embed, dynamics, W_ih0, W_hh0, b_ih0, b_hh0,
           W_ih1, W_hh1, b_ih1, b_hh1, fc_W, fc_b, pred_W, pred_b, seq_len,
           _n_steps=SEQ, _static=False, _trace=False):
    embed = np.asarray(embed, dtype=np.float32)
    dynamics = np.asarray(dynamics, dtype=np.float32)
    wd = _prep_weights(np.asarray(W_ih0, np.float32), np.asarray(W_hh0, np.float32),
                       np.asarray(b_ih0, np.float32), np.asarray(b_hh0, np.float32),
                       np.asarray(W_ih1, np.float32), np.asarray(W_hh1, np.float32),
                       np.asarray(b_ih1, np.float32), np.asarray(b_hh1, np.float32),
                       np.asarray(fc_W, np.float32), np.asarray(fc_b, np.float32),
                       np.asarray(pred_W, np.float32), np.asarray(pred_b, np.float32))

    key = (_n_steps, _static)
    if key not in _NC_CACHE:
        _NC_CACHE[key] = build_nc(n_steps=_n_steps, static=_static)
    nc = _NC_CACHE[key]

    in_maps = []
    for c in range(NCORES):
        m = dict(wd)
        m["embed"] = np.ascontiguousarray(embed[c * B:(c + 1) * B])
        m["x0"] = np.ascontiguousarray(dynamics[c * B:(c + 1) * B, 0, :])
        in_maps.append(m)

    res = run_bass_kernel_spmd(nc, in_maps, list(range(NCORES)), trace=False)
    out = np.concatenate([res.results[c]["out"] for c in range(NCORES)], axis=0)
    if _trace:
        kernel.last_exec_time_ns = _bench_exec(nc, in_maps)
    return out


def _bench_exec(nc, in_maps, n_timed=7):
    """Median wall time of the sharded NEFF execution with device-resident
    inputs (the NTFF profiling hook is unavailable under this axon client,
    so time repeated executions instead)."""
    import time

    import jax
    import jax.numpy as jnp
    from jax.sharding import Mesh, NamedSharding, PartitionSpec
    from jax.experimental.shard_map import shard_map

    from concourse import bass2jax, mybir as _mb

    bass2jax.install_neuronx_cc_hook()
    n_cores = len(in_maps)
    partition_name = (nc.partition_id_tensor.name if nc.partition_id_tensor else None)
    in_names, out_names, out_avals, zero_outs = [], [], [], []
    for alloc in nc.m.functions[0].allocations:
        if not isinstance(alloc, _mb.MemoryLocationSet):
            continue
        name = alloc.memorylocations[0].name
        if alloc.kind == "ExternalInput":
            if name != partition_name:
                in_names.append(name)
        elif alloc.kind == "ExternalOutput":
            out_names.append(name)
            shape = tuple(alloc.tensor_shape)
            dtype = _mb.dt.np(alloc.dtype)
            out_avals.append(jax.core.ShapedArray(shape, dtype))
            zero_outs.append(np.zeros(shape, dtype))
    n_params = len(in_names)
    all_names = list(in_names) + out_names
    if partition_name is not None:
        all_names.append(partition_name)

    def _body(*args):
        operands = list(args)
        if partition_name is not None:
            operands.append(bass2jax.partition_id_tensor())
        return tuple(bass2jax._bass_exec_p.bind(
            *operands,
            out_avals=tuple(out_avals),
            in_names=tuple(all_names),
            out_names=tuple(out_names),
            lowering_input_output_aliases=(),
            sim_require_finite=False,
            sim_require_nnan=False,
            nc=nc,
        ))

    devices = jax.devices()[:n_cores]
    mesh = Mesh(np.asarray(devices), ("core",))
    spec = PartitionSpec("core")
    fn = jax.jit(shard_map(
        _body, mesh=mesh,
        in_specs=(spec,) * (n_params + len(out_names)),
        out_specs=(spec,) * len(out_names), check_rep=False))
    sh = NamedSharding(mesh, spec)
    dev_in = [jax.device_put(
        np.concatenate([np.asarray(in_maps[c][nm]) for c in range(n_cores)], axis=0), sh)
        for nm in in_names]
    dev_zo = [jax.device_put(np.concatenate([z] * n_cores, axis=0), sh) for z in zero_outs]
    r = fn(*dev_in, *dev_zo)
    jax.block_until_ready(r)
    times = []
    for _ in range(n_timed):
        t0 = time.perf_counter()
        r = fn(*dev_in, *dev_zo)
        jax.block_until_ready(r)
        times.append(time.perf_counter() - t0)
    return int(min(times) * 1e9)


# revision 30
# speedup vs baseline: 1.0104x; 1.0104x over previous
"""Trainium2 Bass kernel for nn_Decoder: 2-layer GRU decoder, batch-parallel over 8 cores.

v3 design:
  - Shard batch 128 -> 16 rows/core, replicate weights (SBUF-resident).
  - Weight matmuls batch-major (weights are the moving operand, N=512) in
    fp8e4 DoubleRow mode (2 k-tiles per matmul, 0.5 cyc/row) with weights
    pre-scaled by SCALE=64; biases and the step-invariant glob@Wg term are
    accumulated into PSUM by id16 / ones-row matmuls, so the only
    activation-scale fixup is scale=1/SCALE inside sigmoid/tanh.
  - Gate math runs in TRANSPOSED space (gates on 128 partitions): gate psums
    [16,512] are copied to SBUF fp16 (spread over ACT/DVE/Pool), transposed
    on the PE (is_transpose), and the elementwise GRU update runs on
    [128, 64] tiles. The updated hidden state is produced directly in the
    transposed layout needed as the next matmul's stationary operand - the
    recurrence loop contains no DMA except the per-step output row.
  - fp32 transposed master states; fp8 shadows for the gate matmuls; fp16
    shadow of h1 for the (unscaled) fc head.
  - Software-pipelined emission per step so the in-order PE queue never
    waits long on the vector engines.
"""
import sys

sys.path.insert(0, "/opt/trn_rl_repo")
import numpy as np

import concourse.bass as bass
import concourse.mybir as mybir
import concourse.tile as tile
from concourse import bacc
from concourse.bass import ds, ts
from concourse.bass_utils import run_bass_kernel_spmd

F8 = mybir.dt.float8e4
F16 = mybir.dt.float16
F32 = mybir.dt.float32
AF = mybir.ActivationFunctionType
DR = mybir.MatmulPerfMode.DoubleRow

BS, H, D, SEQ = 128, 1024, 128, 256
NCORES = 8
B = BS // NCORES          # 16 rows per core
KH = H // 128             # 8 k-tiles over hidden dim
KP = KH // 2              # 4 k-PAIRS for fp8 DoubleRow
G3 = 3 * H                # 3072 gate cols
EMBED_DIM = 265216
TS_OFF = 3 * H
TS_LEN = SEQ * (H // 2)   # 131072
SCALE = 64.0              # fp8 weight pre-scale
INV = 1.0 / SCALE


def build_nc(n_steps=SEQ, unroll=15, static=False):
    nc = bacc.Bacc()

    d_embed = nc.declare_dram_parameter("embed", [B, EMBED_DIM], F32, isOutput=False)
    d_x0 = nc.declare_dram_parameter("x0", [B, D], F32, isOutput=False)
    d_wx0 = nc.declare_dram_parameter("wx0", [128, G3], F16, isOutput=False)
    d_wg0 = nc.declare_dram_parameter("wg0", [128, KH, G3], F16, isOutput=False)
    d_whh0 = nc.declare_dram_parameter("whh0", [128, KH, G3], F8, isOutput=False)
    d_wih1 = nc.declare_dram_parameter("wih1", [128, KH, G3], F8, isOutput=False)
    d_whh1 = nc.declare_dram_parameter("whh1", [128, KH, G3], F8, isOutput=False)
    d_fct = nc.declare_dram_parameter("fct", [128, KH, D], F16, isOutput=False)
    d_pred = nc.declare_dram_parameter("predt", [128, 4, D], F16, isOutput=False)
    d_s0b = nc.declare_dram_parameter("s0b", [G3], F16, isOutput=False)
    d_s1b = nc.declare_dram_parameter("s1b", [G3], F16, isOutput=False)
    d_bhh0n = nc.declare_dram_parameter("bhh0n", [H], F16, isOutput=False)
    d_bhh1n = nc.declare_dram_parameter("bhh1n", [H], F16, isOutput=False)
    d_fcb = nc.declare_dram_parameter("fcb", [D, 1], F32, isOutput=False)
    d_pb2 = nc.declare_dram_parameter("pb2", [D], F32, isOutput=False)
    d_id16 = nc.declare_dram_parameter("id16", [B, B], F16, isOutput=False)
    d_id128 = nc.declare_dram_parameter("id128", [128, 128], F16, isOutput=False)
    d_ones = nc.declare_dram_parameter("ones16", [1, B], F16, isOutput=False)
    d_out = nc.declare_dram_parameter("out", [B, SEQ, D], F32, isOutput=True)

    # DRAM scratch for init-time transposes and the tail
    d_bh0 = nc.dram_tensor("bh0", [B, H], F16)
    d_bh1 = nc.dram_tensor("bh1", [B, H], F16)
    d_bg = nc.dram_tensor("bg", [B, H], F16)
    d_bx = nc.dram_tensor("bx", [B, D], F16)
    d_bts = nc.dram_tensor("bts", [128, 512], F16)

    def bcast(ap_1d, parts, n):
        return bass.AP(tensor=ap_1d.tensor, offset=ap_1d.offset,
                       ap=[[0, parts]] + list(ap_1d.ap))

    with tile.TileContext(nc) as tc:
        with (
            tc.tile_pool(name="persist", bufs=1) as pp,
            tc.tile_pool(name="tmp", bufs=2) as tp,
            tc.tile_pool(name="psum", bufs=8, space="PSUM") as qq,
        ):
            # ---------------- resident tiles ----------------
            s_wx0 = pp.tile([128, G3], F16)
            s_whh0 = pp.tile([128, KH, G3], F8)
            s_wih1 = pp.tile([128, KH, G3], F8)
            s_whh1 = pp.tile([128, KH, G3], F8)
            s_fct = pp.tile([128, KH, D], F16)
            s_pred = pp.tile([128, 4, D], F16)
            s_s0 = pp.tile([B, G3], F16)      # (glob@Wg + biases) * SCALE
            s_s1 = pp.tile([B, G3], F16)      # L1 biases (r,z,n(ih)) * SCALE
            s_bh0r = pp.tile([1, H], F16)     # b_hh0 n-part row * SCALE
            s_bh1r = pp.tile([1, H], F16)
            s_ones = pp.tile([1, B], F16)
            s_fcb = pp.tile([D, 1], F32)
            s_pb2 = pp.tile([128, D], F32)
            s_id16 = pp.tile([B, B], F16)
            s_id128 = pp.tile([128, 128], F16)
            # transposed states
            s_h0tm = pp.tile([128, KH, B], F32)   # master
            s_h1tm = pp.tile([128, KH, B], F32)
            s_h0t8 = pp.tile([128, KH, B], F8)    # matmul operands
            s_h1t8 = pp.tile([128, KH, B], F8)
            s_h1t16 = pp.tile([128, KH, B], F16)  # fc operand (unscaled fp16)
            s_xt = pp.tile([128, B], F16)
            s_xo = pp.tile([B, D], F32)

            nc.sync.dma_start(out=s_wx0, in_=d_wx0[:, :])
            nc.sync.dma_start(out=s_whh0, in_=d_whh0[:, :, :])
            nc.sync.dma_start(out=s_wih1, in_=d_wih1[:, :, :])
            nc.sync.dma_start(out=s_whh1, in_=d_whh1[:, :, :])
            nc.sync.dma_start(out=s_fct, in_=d_fct[:, :, :])
            nc.sync.dma_start(out=s_pred, in_=d_pred[:, :, :])
            nc.gpsimd.dma_start(out=s_s0, in_=bcast(d_s0b[:], B, G3))
            nc.gpsimd.dma_start(out=s_s1, in_=bcast(d_s1b[:], B, G3))
            nc.sync.dma_start(out=s_bh0r, in_=d_bhh0n.rearrange("h -> () h"))
            nc.sync.dma_start(out=s_bh1r, in_=d_bhh1n.rearrange("h -> () h"))
            nc.sync.dma_start(out=s_ones, in_=d_ones[:, :])
            nc.sync.dma_start(out=s_fcb, in_=d_fcb[:, :])
            nc.gpsimd.dma_start(out=s_pb2, in_=bcast(d_pb2[:], 128, D))
            nc.sync.dma_start(out=s_id16, in_=d_id16[:, :])
            nc.sync.dma_start(out=s_id128, in_=d_id128[:, :])

            # ---- initial transposed states (fp16 via DRAM bounce) ----
            s_hi0 = tp.tile([B, H], F32, tag="bigtmp")
            nc.sync.dma_start(out=s_hi0, in_=d_embed[:, H:2 * H])
            s_hi0h = tp.tile([B, H], F16, tag="bigtmp")
            nc.scalar.activation(s_hi0h, s_hi0, AF.Copy)
            nc.sync.dma_start(out=d_bh0[:, :], in_=s_hi0h)
            s_hi1 = tp.tile([B, H], F32, tag="bigtmp")
            nc.sync.dma_start(out=s_hi1, in_=d_embed[:, 2 * H:3 * H])
            s_hi1h = tp.tile([B, H], F16, tag="bigtmp")
            nc.scalar.activation(s_hi1h, s_hi1, AF.Copy)
            nc.sync.dma_start(out=d_bh1[:, :], in_=s_hi1h)
            s_h0ti = tp.tile([128, KH, B], F16, tag="bigtmp")
            nc.sync.dma_start_transpose(s_h0ti[:], d_bh0[:, :])
            s_h1ti = tp.tile([128, KH, B], F16, tag="bigtmp")
            nc.sync.dma_start_transpose(s_h1ti[:], d_bh1[:, :])
            nc.vector.tensor_copy(s_h0tm, s_h0ti)
            nc.vector.tensor_copy(s_h1tm, s_h1ti)
            nc.scalar.activation(s_h0t8, s_h0ti, AF.Copy)
            nc.scalar.activation(s_h1t8, s_h1ti, AF.Copy)
            nc.vector.tensor_copy(s_h1t16, s_h1ti)

            # x0 -> xT (fp16)
            s_x0f = tp.tile([B, D], F32, tag="bigtmp")
            nc.sync.dma_start(out=s_x0f, in_=d_x0[:, :])
            s_x0h = tp.tile([B, D], F16, tag="bigtmp")
            nc.scalar.activation(s_x0h, s_x0f, AF.Copy)
            nc.sync.dma_start(out=d_bx[:, :], in_=s_x0h)
            s_xtT = tp.tile([128, 1, B], F16, tag="bigtmp")
            nc.sync.dma_start_transpose(s_xtT[:], d_bx[:, :])
            nc.vector.tensor_copy(s_xt, s_xtT.rearrange("p o b -> p (o b)"))

            # glob -> globT; s_s0 += SCALE * glob @ Wg0 (wg0 pre-scaled)
            s_gf = tp.tile([B, H], F32, tag="wg", bufs=1)
            nc.sync.dma_start(out=s_gf, in_=d_embed[:, 0:H])
            s_gh = tp.tile([B, H], F16, tag="bigtmp")
            nc.scalar.activation(s_gh, s_gf, AF.Copy)
            nc.sync.dma_start(out=d_bg[:, :], in_=s_gh)
            s_gT = tp.tile([128, KH, B], F16, tag="bigtmp")
            nc.sync.dma_start_transpose(s_gT[:], d_bg[:, :])
            NCH = G3 // 512
            pg = [qq.tile([B, 512], F32, tag="ps", name=f"pg{c}") for c in range(NCH)]
            for k in range(KH):
                wbuf = tp.tile([128, G3], F16, tag="wg", bufs=1)
                nc.sync.dma_start(out=wbuf, in_=d_wg0[:, k, :])
                for c in range(NCH):
                    nc.tensor.matmul(pg[c], s_gT[:, k, :], wbuf[:, ts(c, 512)],
                                     start=(k == 0), stop=(k == KH - 1))
            for c in range(NCH):
                nc.vector.tensor_add(s_s0[:, ts(c, 512)], pg[c], s_s0[:, ts(c, 512)])


            # ---------------- one recurrence step ----------------
            # column slices for half h (h in 0,1)
            def slr(h):
                return ts(h, 512)

            def slz(h):
                return slice(H + h * 512, H + (h + 1) * 512)

            def sln(h):
                return slice(2 * H + h * 512, 2 * H + (h + 1) * 512)

            def dr_chain(p, sht8, w, colsl, start, stop):
                # fp8 DoubleRow accumulation of w.T @ h over 4 k-pairs
                for kp in range(KP):
                    nc.tensor.matmul(p, sht8[:, 2 * kp:2 * kp + 2, :],
                                     w[:, 2 * kp:2 * kp + 2, colsl],
                                     start=(start and kp == 0),
                                     stop=(stop and kp == KP - 1), perf_mode=DR)

            def l0_gh(h, tag):
                """gh0 for r,z,ghn of half h (only needs old h0t8)."""
                p_r = qq.tile([B, 512], F32, tag="ps", name=f"{tag}r")
                dr_chain(p_r, s_h0t8, s_whh0, slr(h), True, False)
                p_z = qq.tile([B, 512], F32, tag="ps", name=f"{tag}z")
                dr_chain(p_z, s_h0t8, s_whh0, slz(h), True, False)
                p_gh = qq.tile([B, 512], F32, tag="ps", name=f"{tag}gh")
                dr_chain(p_gh, s_h0t8, s_whh0, sln(h), True, False)
                nc.tensor.matmul(p_gh, s_ones, s_bh0r[:, ts(h, 512)],
                                 start=False, stop=True)
                return p_r, p_z, p_gh

            def l0_gix(h, p_r, p_z, tag):
                """x@Wx + S0 contributions of half h (needs s_xt)."""
                nc.tensor.matmul(p_r, s_xt, s_wx0[:, slr(h)], start=False, stop=False)
                nc.tensor.matmul(p_r, s_id16, s_s0[:, slr(h)], start=False, stop=True)
                nc.tensor.matmul(p_z, s_xt, s_wx0[:, slz(h)], start=False, stop=False)
                nc.tensor.matmul(p_z, s_id16, s_s0[:, slz(h)], start=False, stop=True)
                p_gi = qq.tile([B, 512], F32, tag="ps", name=f"{tag}gi")
                nc.tensor.matmul(p_gi, s_xt, s_wx0[:, sln(h)], start=True, stop=False)
                nc.tensor.matmul(p_gi, s_id16, s_s0[:, sln(h)], start=False, stop=True)
                return p_gi

            def l1_gh(h, tag):
                """gh1 for r,z,ghn of half h (only needs old h1t8)."""
                p_r = qq.tile([B, 512], F32, tag="ps", name=f"{tag}r")
                dr_chain(p_r, s_h1t8, s_whh1, slr(h), True, False)
                p_z = qq.tile([B, 512], F32, tag="ps", name=f"{tag}z")
                dr_chain(p_z, s_h1t8, s_whh1, slz(h), True, False)
                p_gh = qq.tile([B, 512], F32, tag="ps", name=f"{tag}gh")
                dr_chain(p_gh, s_h1t8, s_whh1, sln(h), True, False)
                nc.tensor.matmul(p_gh, s_ones, s_bh1r[:, ts(h, 512)],
                                 start=False, stop=True)
                return p_r, p_z, p_gh

            def l1_gi(h, p_r, p_z, tag):
                """c0@Wih1 + S1 contributions of half h (needs NEW h0t8)."""
                dr_chain(p_r, s_h0t8, s_wih1, slr(h), False, False)
                nc.tensor.matmul(p_r, s_id16, s_s1[:, slr(h)], start=False, stop=True)
                dr_chain(p_z, s_h0t8, s_wih1, slz(h), False, False)
                nc.tensor.matmul(p_z, s_id16, s_s1[:, slz(h)], start=False, stop=True)
                p_gi = qq.tile([B, 512], F32, tag="ps", name=f"{tag}gi")
                dr_chain(p_gi, s_h0t8, s_wih1, sln(h), True, False)
                nc.tensor.matmul(p_gi, s_id16, s_s1[:, sln(h)], start=False, stop=True)
                return p_gi

            def post(h, p_r, p_z, p_gi, p_gh, s_htm, s_ht8, extra16):
                """psum->sbuf copies, PE transposes, transposed gate math for
                half h of one layer. Updates master + fp8 (+fp16) states."""
                # psum -> sbuf fp16 copies, split ACT/DVE
                c_r = tp.tile([B, 512], F16, tag="cp", bufs=8, name="c_r")
                c_z = tp.tile([B, 512], F16, tag="cp", bufs=8, name="c_z")
                c_gi = tp.tile([B, 512], F16, tag="cp", bufs=8, name="c_gi")
                c_gh = tp.tile([B, 512], F16, tag="cp", bufs=8, name="c_gh")
                nc.vector.tensor_copy(c_r, p_r)
                nc.scalar.activation(c_z, p_z, AF.Copy)
                nc.vector.tensor_copy(c_gi, p_gi)
                nc.scalar.activation(c_gh, p_gh, AF.Copy)
                # PE transposes: [16,128] chunks -> [128,16]; kinds r,z,gi,gh
                pT = qq.tile([128, 4, 4, B], F16, tag="ps", name="pT")
                for ki, src in enumerate((c_r, c_z, c_gi, c_gh)):
                    for q in range(4):
                        nc.tensor.transpose(pT[:, ki, q, :],
                                            src[:, q * 128:(q + 1) * 128], s_id16)
                # transposed gate math on [128, 4*B] tiles
                rz = tp.tile([128, 2, 4, B], F16, tag="rz", bufs=4)
                nc.scalar.activation(rz, pT[:, 0:2], AF.Sigmoid, scale=INV)
                tn = tp.tile([128, 4, B], F32, tag="tn", bufs=4)
                nc.vector.tensor_mul(tn, rz[:, 0], pT[:, 3])
                nc.vector.tensor_add(tn, tn, pT[:, 2])
                nc.scalar.activation(tn, tn, AF.Tanh, scale=INV)
                m = s_htm[:, 4 * h:4 * h + 4, :]
                td = tp.tile([128, 4, B], F32, tag="td", bufs=4)
                nc.vector.tensor_sub(td, m, tn)
                nc.vector.tensor_mul(td, rz[:, 1], td)
                nc.vector.tensor_add(m, tn, td)
                nc.scalar.activation(s_ht8[:, 4 * h:4 * h + 4, :], m, AF.Copy)
                if extra16 is not None:
                    nc.vector.tensor_copy(extra16[:, 4 * h:4 * h + 4, :], m)

            def fc_block(t_expr):
                pfcT = qq.tile([D, B], F32, tag="ps", name="pfcT")
                for k in range(KH):
                    nc.tensor.matmul(pfcT, s_fct[:, k, :], s_h1t16[:, k, :],
                                     start=(k == 0), stop=(k == KH - 1))
                nc.scalar.activation(s_xt, pfcT, AF.Sigmoid, bias=s_fcb[:, :])
                # output row = s_xt transposed (already sigmoid'd)
                pxo = qq.tile([B, D], F16, tag="ps", name="pxo")
                nc.tensor.transpose(pxo, s_xt, s_id128)
                nc.vector.tensor_copy(s_xo, pxo)
                nc.sync.dma_start(out=d_out[:, ds(t_expr, 1), :],
                                  in_=s_xo.rearrange("b d -> b () d"))

            def step(t_expr, first):
                # L0 gh matmuls (old h0) keep the PE busy while the previous
                # step's L1 gate math finishes on the vector engines.
                r0a, z0a, gh0a = l0_gh(0, "a")
                r0b, z0b, gh0b = l0_gh(1, "b")
                # previous step's fc -> s_xt for this step + output row
                if not first:
                    fc_block(t_expr - 1)
                gi0a = l0_gix(0, r0a, z0a, "a")
                gi0b = l0_gix(1, r0b, z0b, "b")
                # L1 gh matmuls (old h1) cover the L0 copies/gate math
                r1a, z1a, gh1a = l1_gh(0, "c")
                r1b, z1b, gh1b = l1_gh(1, "d")
                post(0, r0a, z0a, gi0a, gh0a, s_h0tm, s_h0t8, None)
                post(1, r0b, z0b, gi0b, gh0b, s_h0tm, s_h0t8, None)
                # L1 gi matmuls contract the NEW h0t8
                gi1a = l1_gi(0, r1a, z1a, "c")
                gi1b = l1_gi(1, r1b, z1b, "d")
                post(0, r1a, z1a, gi1a, gh1a, s_h1tm, s_h1t8, s_h1t16)
                post(1, r1b, z1b, gi1b, gh1b, s_h1tm, s_h1t8, s_h1t16)

            if static:
                for t in range(n_steps):
                    step(t, t == 0)
            else:
                step(0, True)
                while (n_steps - 1) % unroll != 0:
                    unroll -= 1
                with tc.For_i(1, n_steps, unroll,
                              hint_engines=(mybir.EngineType.PE,)) as iv:
                    for j in range(unroll):
                        step(iv + j, False)
            fc_block(n_steps - 1)

            # ---------------- tail: trend/season + residual ----------------
            for b in range(B):
                for si in range(2):
                    base = TS_OFF + si * 128 * 512
                    ps_o = qq.tile([128, D], F32, tag="ps")
                    for which in range(2):  # 0=trend 1=season
                        off = base + which * TS_LEN
                        src = d_embed[b:b + 1, off:off + 65536].rearrange(
                            "o (s f) -> (o s) f", f=512)
                        t_f = tp.tile([128, 512], F32, tag="tsf")
                        nc.sync.dma_start(out=t_f, in_=src)
                        t_h = tp.tile([128, 512], F16, tag="bigtmp")
                        nc.scalar.activation(t_h, t_f, AF.Copy)
                        # PE transpose [128,128] chunks (f on partitions)
                        p_tT = qq.tile([128, 4, 128], F16, tag="ps", name="p_tT")
                        for jj in range(4):
                            nc.tensor.transpose(p_tT[:, jj, :],
                                                t_h[:, jj * 128:(jj + 1) * 128],
                                                s_id128)
                        t_T = tp.tile([128, 4, 128], F16, tag="bigtmp")
                        nc.vector.tensor_copy(t_T, p_tT)
                        for jj in range(4):
                            nc.tensor.matmul(ps_o, t_T[:, jj, :], s_pred[:, jj, :],
                                             start=(which == 0 and jj == 0),
                                             stop=(which == 1 and jj == 3))
                    r_c = tp.tile([128, D], F32, tag="bigtmp")
                    nc.sync.dma_start(out=r_c, in_=d_out[b, si * 128:(si + 1) * 128, :])
                    nc.vector.tensor_add(r_c, ps_o, r_c)
                    nc.vector.tensor_add(r_c, r_c, s_pb2)
                    nc.sync.dma_start(out=d_out[b, si * 128:(si + 1) * 128, :], in_=r_c)

    nc.compile()
    return nc


def _prep_weights(W_ih0, W_hh0, b_ih0, b_hh0, W_ih1, W_hh1, b_ih1, b_hh1,
                  fc_W, fc_b, pred_W, pred_b):
    f16 = np.float16
    f8 = mybir.dt.np(F8)

    def karr(WT, dt, scale=1.0):  # [K, N] -> [128, K/128, N]
        K, N = WT.shape
        return np.ascontiguousarray(
            (WT * scale).reshape(K // 128, 128, N).transpose(1, 0, 2)).astype(dt)

    return dict(
        wx0=(np.ascontiguousarray(W_ih0[:, H:H + D].T) * SCALE).astype(f16),
        wg0=karr(W_ih0[:, :H].T, f16, SCALE),
        whh0=karr(W_hh0.T, f8, SCALE),
        wih1=karr(W_ih1.T, f8, SCALE),
        whh1=karr(W_hh1.T, f8, SCALE),
        fct=karr(fc_W.T, f16),
        predt=np.ascontiguousarray(
            pred_W.T.reshape(4, 128, D).transpose(1, 0, 2)).astype(f16),
        s0b=(np.concatenate([(b_ih0 + b_hh0)[:2 * H], b_ih0[2 * H:]]) * SCALE).astype(f16),
        s1b=(np.concatenate([(b_ih1 + b_hh1)[:2 * H], b_ih1[2 * H:]]) * SCALE).astype(f16),
        bhh0n=(b_hh0[2 * H:] * SCALE).astype(f16),
        bhh1n=(b_hh1[2 * H:] * SCALE).astype(f16),
        fcb=np.ascontiguousarray(fc_b.reshape(D, 1)).astype(np.float32),
        id16=np.eye(B, dtype=np.float16),
        id128=np.eye(128, dtype=np.float16),
        ones16=np.ones((1, B), dtype=np.float16),
        pb2=(2.0 * pred_b).astype(np.float32),
    )


_NC_CACHE = {}


def kernel(embed, dynamics, W_ih0, W_hh0, b_ih0, b_hh0,
           W_ih1, W_hh1, b_ih1, b_hh1, fc_W, fc_b, pred_W, pred_b, seq_len,
           _n_steps=SEQ, _static=False, _trace=False):
    embed = np.asarray(embed, dtype=np.float32)
    dynamics = np.asarray(dynamics, dtype=np.float32)
    wd = _prep_weights(np.asarray(W_ih0, np.float32), np.asarray(W_hh0, np.float32),
                       np.asarray(b_ih0, np.float32), np.asarray(b_hh0, np.float32),
                       np.asarray(W_ih1, np.float32), np.asarray(W_hh1, np.float32),
                       np.asarray(b_ih1, np.float32), np.asarray(b_hh1, np.float32),
                       np.asarray(fc_W, np.float32), np.asarray(fc_b, np.float32),
                       np.asarray(pred_W, np.float32), np.asarray(pred_b, np.float32))

    key = (_n_steps, _static)
    if key not in _NC_CACHE:
        _NC_CACHE[key] = build_nc(n_steps=_n_steps, static=_static)
    nc = _NC_CACHE[key]

    in_maps = []
    for c in range(NCORES):
        m = dict(wd)
        m["embed"] = np.ascontiguousarray(embed[c * B:(c + 1) * B])
        m["x0"] = np.ascontiguousarray(dynamics[c * B:(c + 1) * B, 0, :])
        in_maps.append(m)

    res = run_bass_kernel_spmd(nc, in_maps, list(range(NCORES)), trace=False)
    out = np.concatenate([res.results[c]["out"] for c in range(NCORES)], axis=0)
    if _trace:
        kernel.last_exec_time_ns = _bench_exec(nc, in_maps)
    return out


def _bench_exec(nc, in_maps, n_timed=7):
    """Median wall time of the sharded NEFF execution with device-resident
    inputs (the NTFF profiling hook is unavailable under this axon client,
    so time repeated executions instead)."""
    import time

    import jax
    import jax.numpy as jnp
    from jax.sharding import Mesh, NamedSharding, PartitionSpec
    from jax.experimental.shard_map import shard_map

    from concourse import bass2jax, mybir as _mb

    bass2jax.install_neuronx_cc_hook()
    n_cores = len(in_maps)
    partition_name = (nc.partition_id_tensor.name if nc.partition_id_tensor else None)
    in_names, out_names, out_avals, zero_outs = [], [], [], []
    for alloc in nc.m.functions[0].allocations:
        if not isinstance(alloc, _mb.MemoryLocationSet):
            continue
        name = alloc.memorylocations[0].name
        if alloc.kind == "ExternalInput":
            if name != partition_name:
                in_names.append(name)
        elif alloc.kind == "ExternalOutput":
            out_names.append(name)
            shape = tuple(alloc.tensor_shape)
            dtype = _mb.dt.np(alloc.dtype)
            out_avals.append(jax.core.ShapedArray(shape, dtype))
            zero_outs.append(np.zeros(shape, dtype))
    n_params = len(in_names)
    all_names = list(in_names) + out_names
    if partition_name is not None:
        all_names.append(partition_name)

    def _body(*args):
        operands = list(args)
        if partition_name is not None:
            operands.append(bass2jax.partition_id_tensor())
        return tuple(bass2jax._bass_exec_p.bind(
            *operands,
            out_avals=tuple(out_avals),
            in_names=tuple(all_names),
            out_names=tuple(out_names),
            lowering_input_output_aliases=(),
            sim_require_finite=False,
            sim_require_nnan=False,
            nc=nc,
        ))

    devices = jax.devices()[:n_cores]
    mesh = Mesh(np.asarray(devices), ("core",))
    spec = PartitionSpec("core")
    fn = jax.jit(shard_map(
        _body, mesh=mesh,
        in_specs=(spec,) * (n_params + len(out_names)),
        out_specs=(spec,) * len(out_names), check_rep=False))
    sh = NamedSharding(mesh, spec)
    dev_in = [jax.device_put(
        np.concatenate([np.asarray(in_maps[c][nm]) for c in range(n_cores)], axis=0), sh)
        for nm in in_names]
    dev_zo = [jax.device_put(np.concatenate([z] * n_cores, axis=0), sh) for z in zero_outs]
    r = fn(*dev_in, *dev_zo)
    jax.block_until_ready(r)
    times = []
    for _ in range(n_timed):
        t0 = time.perf_counter()
        r = fn(*dev_in, *dev_zo)
        jax.block_until_ready(r)
        times.append(time.perf_counter() - t0)
    return int(min(times) * 1e9)
